# revision 18
# baseline (speedup 1.0000x reference)
"""DeepONet-style neural operator forward pass on 8 TRN2 NeuronCores.

Pure data parallel over the batch (131072 rows -> 16384/core), weights
replicated. Activations live feature-major ([feat, rows]); rows are
processed in blocks of 512 (one fp32 PSUM bank).

This version runs the heavy layers in fp8e4m3 with DoubleRow matmuls
(0.5 PE cycles/row, 2x the fp16 rate): L1 544->1024, L2 1024->512,
L3 512->256, trunk 256->256 and the output heads. Layout: activations are
kept as "pair" tiles [128, 2, N] so a 256-deep contraction is one
DoubleRow matmul. Host supplies a fp16 feature matrix stac16 [rows, 128]
(pos, ones, state, action, pos^2 columns) that is DMA-transposed (XBAR)
straight into SBUF, eliminating all PE input transposes; sensor dist^2 is
one K=21 fp16 matmul (|s|^2 and |pos|^2 folded in via ones/pos^2 rows),
sqrt runs as fp16 magic-rsqrt + 1 Newton step on the DVE, and exp on ACT.
"""

import numpy as np

import concourse.bass as bass
import concourse.mybir as mybir
import concourse.tile as tile
from concourse import bacc

F32 = mybir.dt.float32
F16 = mybir.dt.float16
E4 = mybir.dt.float8e4
I16 = mybir.dt.int16
I32 = mybir.dt.int32
AF = mybir.ActivationFunctionType
ALU = mybir.AluOpType
AX = mybir.AxisListType
DR = mybir.MatmulPerfMode.DoubleRow

SD = 13          # state dim
AD = 4           # action dim
J = SD + AD      # 17 per-sensor features
NS = 32          # sensors
H1, H2, H4, H8 = 1024, 512, 256, 128
B_FULL = 131072
N_CORES = 8
RPC = B_FULL // N_CORES   # rows per core
NB = 512                  # rows per block (= fp32 PSUM bank)

# stacT row layout: 0-2 pos, 3 ones, 4-13 state[3:], 14-17 action, 18-20 pos^2
ROWMAP = [j if j < 3 else j + 1 for j in range(J)]   # feature j -> stacT row

# engine assignment for the relu after L1 (4 merged m-pairs) and L2 (4 m-tiles)
RELU1_ENG = ["act", "vector", "act", "vector"]
RELU2_ENG = ["vector", "act", "vector", "act"]


def _const_specs():
    b8 = [("w1A", 128, 2048), ("w1B", 128, 2048), ("w1C", 32, 2048)]
    for k in range(4):
        b8.append((f"w2_{k}", 128, 1024))
    for k in range(2):
        b8.append((f"w3_{k}", 128, 512))
    b8 += [("tw2p", 128, 512), ("pwp", 128, 32), ("qw2p", 128, 32)]
    for t in range(4):
        b8.append((f"esel_{t}", 18, 256))
    b8.append(("eselC", 18, 64))
    b16 = [("sl21", 21, 128), ("tw1b", 4, 256), ("qw1b", 4, 128),
           ("ones2n", 16, 2 * NB)]
    bf = [("bb2t", 128, 4), ("bb3t", 128, 2),
          ("tb2t", 128, 2), ("c13", SD, 1), ("rw13", SD, 1),
          ("id13", SD, SD)]

    def offsets(specs):
        out, o = {}, 0
        for name, p, w in specs:
            out[name] = (o, p, w)
            o += w
        return out, o
    o8, w8 = offsets(b8)
    o16, w16 = offsets(b16)
    of, wf = offsets(bf)
    return o8, w8, o16, w16, of, wf


C8, C8W, C16, C16W, CF, CFW = _const_specs()


def build_nc(rpc=RPC, repeats=1):
    assert rpc % NB == 0
    nblk = rpc // NB
    nc = bacc.Bacc(trn_type="TRN2")

    def inp(name, shape, dt=F32):
        return nc.dram_tensor(name, shape, dt, kind="ExternalInput").ap()

    state = inp("state", [rpc, SD])
    stac16 = inp("stac16", [rpc, 128], F16)
    blob8 = inp("blob8", [128, C8W], E4)
    blob16 = inp("blob16", [128, C16W], F16)
    blobf = inp("blobf", [128, CFW])

    out = nc.dram_tensor("out", [rpc, SD], F32, kind="ExternalOutput").ap()

    with tile.TileContext(nc) as tc:
        for _rep in range(repeats):
            _body(tc, nblk, locals())
    nc.compile()
    return nc


def _body(tc, nblk, t):
    nc = tc.nc

    import contextlib
    stack = contextlib.ExitStack()
    consts = stack.enter_context(tc.tile_pool(name="consts", bufs=1))
    sb_in = stack.enter_context(tc.tile_pool(name="sb_in", bufs=1))
    sb_act = stack.enter_context(tc.tile_pool(name="sb_act", bufs=1))
    sb_sm = stack.enter_context(tc.tile_pool(name="sb_sm", bufs=1))
    ps_mm = stack.enter_context(tc.tile_pool(name="ps_mm", bufs=4, space="PSUM"))
    ps_pair = stack.enter_context(tc.tile_pool(name="ps_pair", bufs=2, space="PSUM"))

    blob8_sb = consts.tile([128, C8W], E4, name="blob8_sb", tag="blob8_sb")
    blob16_sb = consts.tile([128, C16W], F16, name="blob16_sb", tag="blob16_sb")
    blobf_sb = consts.tile([128, CFW], F32, name="blobf_sb", tag="blobf_sb")
    NCH = 8
    step = (C8W + NCH - 1) // NCH
    for i in range(NCH):
        a, b = i * step, min((i + 1) * step, C8W)
        nc.sync.dma_start(out=blob8_sb[:, a:b], in_=t["blob8"][:, a:b])
    nc.sync.dma_start(out=blob16_sb, in_=t["blob16"])
    nc.sync.dma_start(out=blobf_sb, in_=t["blobf"])

    def v8(name):
        o, p, w = C8[name]
        return blob8_sb[0:p, o:o + w]

    def v16(name):
        o, p, w = C16[name]
        return blob16_sb[0:p, o:o + w]

    def vf(name):
        o, p, w = CF[name]
        return blobf_sb[0:p, o:o + w]

    def pair_view(name, m=None, mw=None):
        o, p, w = C8[name]
        r = blob8_sb[0:p, o:o + w].rearrange("p (i m) -> p i m", i=2)
        if m is None:
            return r
        return r[:, :, m * mw:(m + 1) * mw]

    w1A = [pair_view("w1A", m, 128) for m in range(8)]
    w1B = [pair_view("w1B", m, 128) for m in range(8)]
    w1C = [pair_view("w1C", m, 128) for m in range(8)]
    ones2n = v16("ones2n")
    w2p = [[pair_view(f"w2_{k}", m, 128) for m in range(4)] for k in range(4)]
    w3p = [[pair_view(f"w3_{k}", m, 128) for m in range(2)] for k in range(2)]
    tw2p = [pair_view("tw2p", m, 128) for m in range(2)]
    pwp = pair_view("pwp")
    qw2p = pair_view("qw2p")
    esel8 = [pair_view(f"esel_{t_}") for t_ in range(4)]
    eselC = pair_view("eselC")
    sl21 = v16("sl21")
    tw1b = v16("tw1b")
    qw1b = v16("qw1b")
    bb2sb = vf("bb2t")
    bb3sb = vf("bb3t")
    tb2sb = vf("tb2t")
    c13sb = vf("c13")
    rw13sb = vf("rw13")
    id13sb = vf("id13")
    zero1 = consts.tile([128, 1], F32)
    nc.vector.memset(zero1, 0.0)

    state, stac16, outdr = t["state"], t["stac16"], t["out"]

    def relu_op(eng, out, ps, bias_col):
        if eng == "act":
            nc.scalar.activation(out=out, in_=ps, func=AF.Relu,
                                 bias=bias_col, scale=1.0)
        else:
            e = nc.gpsimd if eng == "pool" else nc.vector
            e.tensor_scalar(out=out, in0=ps, scalar1=bias_col, scalar2=0.0,
                            op0=ALU.add, op1=ALU.max)

    ablk = {}

    def stage_a(blk):
        r0 = blk * NB
        st_ac = sb_in.tile([128, 4, SD], F32, tag="st_ac", bufs=3)
        nc.sync.dma_start(
            out=st_ac,
            in_=state[r0:r0 + NB, :].rearrange("(c p) d -> p c d", p=128))
        stacT = sb_in.tile([128, NB], F16, tag="stacT", bufs=3)
        nc.sync.dma_start(out=stacT, in_=stac16[r0:r0 + NB, :],
                          transpose=True)

        # q = dist^2 (K=21 fp16 matmul; |s|^2, |pos|^2 folded via const rows)
        q_ps = ps_mm.tile([128, NB], F32, tag="mm", bufs=4)
        nc.tensor.matmul(q_ps, sl21, stacT[0:21, :], start=True, stop=True)
        qs = sb_sm.tile([128, NB], F16, tag="qs", bufs=3)
        nc.scalar.activation(out=qs, in_=q_ps, func=AF.Relu,
                             bias=zero1[:, 0:1], scale=1.0)

        # dist = q * rsqrt(q): fp16 magic seed + 1 Newton (gpsimd, SBUF only)
        r = sb_sm.tile([128, NB], F16, tag="r", bufs=3)
        y = sb_sm.tile([128, NB], F16, tag="y", bufs=3)
        u = sb_sm.tile([128, NB], F16, tag="u", bufs=3)
        nc.vector.tensor_scalar(
            out=r.bitcast(I16), in0=qs.bitcast(I16), scalar1=1, scalar2=None,
            op0=ALU.logical_shift_right)
        nc.vector.tensor_scalar(
            out=r.bitcast(I16), in0=r.bitcast(I16), scalar1=-1,
            scalar2=0x59BA, op0=ALU.mult, op1=ALU.add)
        nc.gpsimd.tensor_mul(y, qs, r)
        nc.gpsimd.tensor_mul(u, y, r)
        nc.gpsimd.tensor_scalar(out=u, in0=u, scalar1=-0.5, scalar2=1.5,
                                op0=ALU.mult, op1=ALU.add)
        nc.gpsimd.tensor_mul(y, y, u)   # y = dist

        w16 = sb_in.tile([128, NB], F16, tag="w16", bufs=3)
        nc.scalar.activation(out=w16, in_=y, func=AF.Exp,
                             bias=zero1[:, 0:1], scale=-2.0)

        # wrepC[p, i, n] = w16[i*16+p, n]; rows 16-31 = ones (bias + pad)
        wrepC = sb_in.tile([32, 2, NB], F16, tag="wrepC", bufs=3)
        nc.sync.dma_start(out=wrepC[0:16, 0, :], in_=w16[0:16, :])
        nc.sync.dma_start(out=wrepC[0:16, 1, :], in_=w16[16:32, :])
        nc.sync.dma_start(out=wrepC[16:32, :, :],
                          in_=ones2n.rearrange("p (i n) -> p i n", i=2))

        stac8 = sb_in.tile([18, NB], E4, tag="stac8", bufs=3)
        nc.gpsimd.tensor_copy(stac8, stacT[0:18, :])
        s8b = stac8[:, None, :].broadcast_to([18, 2, NB])

        # replicate features to enc channels (DoubleRow esel matmuls), cast
        # psum->fp16 via gpsimd DGE DMA, multiply by sensor weights on
        # gpsimd -> fp8 enc pair tiles
        enc = []
        w16b = w16[:, None, :].broadcast_to([128, 2, NB])
        srepC = ps_pair.tile([128, 2, NB], F32, tag="pair", bufs=2)
        nc.tensor.matmul(srepC[0:32, 0, :], eselC, s8b, start=True,
                         stop=True, perf_mode=DR)
        nc.tensor.matmul(srepC[0:32, 1, :], eselC, s8b, start=True,
                         stop=True, perf_mode=DR)
        etC = sb_in.tile([32, 2, NB], E4, tag="encC", bufs=3)
        nc.vector.tensor_mul(etC, srepC[0:32, :, :], wrepC)
        for p_ in range(2):
            srep = ps_pair.tile([128, 2, NB], F32, tag="pair", bufs=2)
            nc.tensor.matmul(srep[:, 0, :], esel8[2 * p_], s8b, start=True,
                             stop=True, perf_mode=DR)
            nc.tensor.matmul(srep[:, 1, :], esel8[2 * p_ + 1], s8b,
                             start=True, stop=True, perf_mode=DR)
            et = sb_in.tile([128, 2, NB], E4, tag=f"enc{p_}", bufs=3)
            nc.vector.tensor_mul(et, srep, w16b)
            enc.append(et)
        enc.append(etC)
        ablk[blk] = dict(st_ac=st_ac, stacT=stacT, enc=enc)

    def stage_b(blk):
        st = ablk[blk]
        enc, stacT = st["enc"], st["stacT"]

        # ---- branch L1: 544 -> 1024, bias pre-added via the ones channel;
        # two m-tiles share one [128, 2, NB] psum -> single merged relu ----
        h1 = [sb_act.tile([128, 2, NB], E4, tag=f"h1_{k}", bufs=2,
                          name=f"h1_{k}") for k in range(4)]
        for p_ in range(4):
            ps = ps_pair.tile([128, 2, NB], F32, tag="pair", bufs=2)
            for i in range(2):
                m = 2 * p_ + i
                nc.tensor.matmul(ps[:, i, :], w1A[m], enc[0], start=True,
                                 stop=False, perf_mode=DR)
                nc.tensor.matmul(ps[:, i, :], w1B[m], enc[1], start=False,
                                 stop=False, perf_mode=DR)
                nc.tensor.matmul(ps[:, i, :], w1C[m], enc[2], start=False,
                                 stop=True, perf_mode=DR)
            eng = RELU1_ENG[p_]
            if eng == "act":
                nc.scalar.activation(out=h1[p_], in_=ps, func=AF.Relu,
                                     bias=zero1[:, 0:1], scale=1.0)
            else:
                nc.vector.tensor_scalar_max(h1[p_], ps, 0.0)

        # ---- branch L2: 1024 -> 512 (4 DoubleRow matmuls per m-tile) ----
        h2 = [sb_act.tile([128, 2, NB], E4, tag=f"h2_{k}", bufs=2,
                          name=f"h2_{k}") for k in range(2)]
        for m in range(4):
            ps = ps_mm.tile([128, NB], F32, tag="mm", bufs=4)
            for k in range(4):
                nc.tensor.matmul(ps, w2p[k][m], h1[k], start=(k == 0),
                                 stop=(k == 3), perf_mode=DR)
            relu_op(RELU2_ENG[m], h2[m // 2][:, m % 2, :], ps,
                    bb2sb[:, m:m + 1])

        # ---- trunk: tanh(pos@tw1+tb1) [bias folded], tanh(.@tw2+tb2) ----
        tt_ps = ps_pair.tile([128, 2, NB], F32, tag="pair", bufs=2)
        nc.tensor.matmul(tt_ps[:, 0, :], tw1b[:, 0:128], stacT[0:4, :],
                         start=True, stop=True)
        nc.tensor.matmul(tt_ps[:, 1, :], tw1b[:, 128:256], stacT[0:4, :],
                         start=True, stop=True)
        tt8 = sb_act.tile([128, 2, NB], E4, tag="tt8", bufs=2)
        nc.scalar.activation(out=tt8, in_=tt_ps, func=AF.Tanh,
                             bias=zero1[:, 0:1], scale=1.0)
        trunk = []
        for m in range(2):
            ps = ps_mm.tile([128, NB], F32, tag="mm", bufs=4)
            nc.tensor.matmul(ps, tw2p[m], tt8, start=True, stop=True,
                             perf_mode=DR)
            tm = sb_act.tile([128, NB], F16, tag="trunk", bufs=3)
            nc.scalar.activation(out=tm, in_=ps, func=AF.Tanh,
                                 bias=tb2sb[:, m:m + 1], scale=1.0)
            trunk.append(tm)

        # ---- qnet hidden: relu(pos@qw1+qb1) [bias folded] ----
        ps = ps_mm.tile([128, NB], F32, tag="mm", bufs=4)
        nc.tensor.matmul(ps, qw1b, stacT[0:4, :], start=True, stop=True)
        bq8 = sb_act.tile([128, NB], E4, tag="bq8", bufs=2)
        nc.scalar.activation(out=bq8, in_=ps, func=AF.Relu,
                             bias=zero1[:, 0:1], scale=1.0)

        # ---- branch L3 (+bias) fused with interaction multiply ----
        inter = sb_act.tile([128, 2, NB], E4, tag="inter", bufs=2)
        for m in range(2):
            ps = ps_mm.tile([128, NB], F32, tag="mm", bufs=4)
            for k in range(2):
                nc.tensor.matmul(ps, w3p[k][m], h2[k], start=(k == 0),
                                 stop=(k == 1), perf_mode=DR)
            nc.vector.scalar_tensor_tensor(
                out=inter[:, m, :], in0=ps, scalar=bb3sb[:, m:m + 1],
                in1=trunk[m], op0=ALU.add, op1=ALU.mult)

        # ---- tail: (inter@pw + bq@qw2) in one psum, DoubleRow ----
        tail_full = ps_mm.tile([128, NB], F32, tag="mm", bufs=4)
        tail_ps = tail_full[0:16, :]
        nc.tensor.matmul(tail_ps, pwp, inter, start=True, stop=False,
                         perf_mode=DR)
        nc.tensor.matmul(tail_ps, qw2p,
                         bq8[:, None, :].broadcast_to([128, 2, NB]),
                         start=False, stop=True, perf_mode=DR)
        combT = sb_sm.tile([SD, NB], F32, tag="combT", bufs=2)
        nc.vector.tensor_scalar(
            out=combT, in0=tail_full[0:SD, :], scalar1=rw13sb[:, 0:1],
            scalar2=c13sb[:, 0:1], op0=ALU.mult, op1=ALU.add)
        ablk[blk]["combT"] = combT

    def stage_c(blk):
        r0 = blk * NB
        st = ablk.pop(blk)
        st_ac, combT = st["st_ac"], st["combT"]
        trt = ps_mm.tile([128, NB], F32, tag="mm", bufs=4)
        nxt = sb_sm.tile([128, 4, SD], F32, tag="nxt", bufs=2)
        sq = sb_sm.tile([128, 4, 4], F32, tag="sq", bufs=2)
        for c in range(4):
            tr_ps = trt[:, 16 * c:16 * c + SD]
            nc.tensor.transpose(tr_ps, combT[:, c * 128:(c + 1) * 128],
                                id13sb)
            nc.vector.tensor_add(nxt[:, c, :], tr_ps, st_ac[:, c, :])
            nc.vector.tensor_mul(sq[:, c, :], nxt[:, c, 3:7], nxt[:, c, 3:7])
        qn = sb_sm.tile([128, 4], F32, tag="qn", bufs=2)
        nc.vector.reduce_sum(out=qn.rearrange("p (c o) -> p c o", o=1),
                             in_=sq, axis=AX.X)
        rq = sb_sm.tile([128, 4], F32, tag="rq", bufs=2)
        uq = sb_sm.tile([128, 4], F32, tag="uq", bufs=2)
        yq = sb_sm.tile([128, 4], F32, tag="yq", bufs=2)
        nc.vector.tensor_scalar(
            out=rq.bitcast(I32), in0=qn.bitcast(I32), scalar1=1, scalar2=None,
            op0=ALU.arith_shift_right)
        nc.vector.tensor_scalar(
            out=rq.bitcast(I32), in0=rq.bitcast(I32), scalar1=-1,
            scalar2=0x5F3759DF, op0=ALU.mult, op1=ALU.add)
        for it in range(2):
            nc.gpsimd.tensor_mul(yq, qn, rq)
            nc.gpsimd.tensor_mul(uq, yq, rq)
            nc.gpsimd.tensor_scalar(out=uq, in0=uq, scalar1=-0.5, scalar2=1.5,
                                    op0=ALU.mult, op1=ALU.add)
            nc.gpsimd.tensor_mul(rq, rq, uq)
        outt = sb_sm.tile([128, 4, SD], F32, tag="outt", bufs=2)
        nc.gpsimd.tensor_copy(outt, nxt)
        for c in range(4):
            nc.gpsimd.tensor_scalar_mul(
                outt[:, c, 3:7], nxt[:, c, 3:7], rq[:, c:c + 1])
        out_dst = outdr[r0:r0 + NB, :].rearrange("(c p) d -> p c d", p=128)
        nc.sync.dma_start(out=out_dst, in_=outt)

    # software-pipelined emission: A two blocks ahead of B/C
    stage_a(0)
    if nblk > 1:
        stage_a(1)
    for blk in range(nblk):
        stage_b(blk)
        stage_c(blk)
        if blk + 2 < nblk:
            stage_a(blk + 2)
    stack.close()


def _host_prep(inputs):
    """Precompute fp8/fp16 weight blobs and the padded feature matrix."""
    import ml_dtypes
    E4NP = ml_dtypes.float8_e4m3
    f = lambda x: np.ascontiguousarray(np.asarray(x, dtype=np.float32))
    sl = f(inputs["sensor_locations"])            # [32, 3]
    pidx = np.arange(128) % NS

    sl21 = np.zeros((21, 128), np.float32)
    sl21[0:3, :] = -2.0 * sl[pidx].T
    sl21[3, :] = np.square(sl).sum(1)[pidx]
    sl21[18:21, :] = 1.0
    tw1b = np.concatenate([f(inputs["tw1"]), f(inputs["tb1"])[None, :]], 0)
    qw1b = np.concatenate([f(inputs["qw1"]), f(inputs["qb1"])[None, :]], 0)

    # enc channel ch = j*32 + s  <-  original bw1 row s*17 + j
    ch = np.arange(544)
    w1p = f(inputs["bw1"])[(ch % NS) * J + ch // NS, :]        # [544, 1024]

    def drpack(w):           # [2*P, M] -> [P, 2, M] -> [P, 2*M]
        p2 = w.shape[0] // 2
        return w.reshape(2, p2, -1).transpose(1, 0, 2).reshape(p2, -1)

    # w1C rows: 16 feature channels + bias channel (bb1 | 0) + zero pad
    w1c = np.zeros((32, 2, H1), np.float32)
    w1c[0:16] = w1p[512:544].reshape(2, 16, H1).transpose(1, 0, 2)
    w1c[16, 0, :] = f(inputs["bb1"])
    c8 = {
        "w1A": drpack(w1p[0:256]), "w1B": drpack(w1p[256:512]),
        "w1C": w1c.reshape(32, 2 * H1),
        "tw2p": drpack(f(inputs["tw2"])),
    }
    w2 = f(inputs["bw2"]); w3 = f(inputs["bw3"])
    for k in range(4):
        c8[f"w2_{k}"] = drpack(w2[k * 256:(k + 1) * 256])
    for k in range(2):
        c8[f"w3_{k}"] = drpack(w3[k * 256:(k + 1) * 256])
    pw = f(inputs["pw"])                                       # [256, 13]
    pwp = drpack(pw)                                           # [128, 26]
    c8["pwp"] = np.concatenate(
        [pwp.reshape(128, 2, SD),
         np.zeros((128, 2, 16 - SD), np.float32)], -1).reshape(128, 32)
    qw2 = f(inputs["qw2"])                                     # [128, 13]
    qw2p = np.stack([qw2, np.zeros_like(qw2)], 1)              # [128, 2, 13]
    c8["qw2p"] = np.concatenate(
        [qw2p, np.zeros((128, 2, 16 - SD), np.float32)], -1).reshape(128, 32)
    for t in range(4):
        e = np.zeros((18, 2, 128), np.float32)
        for m in range(128):
            e[ROWMAP[(128 * t + m) // NS], 0, m] = 1.0
        c8[f"esel_{t}"] = e.reshape(18, 256)
    # eselC: cols 0-15 select the action[3] feature row (17); col 16 selects
    # the ones row (3) -> bias channel for L1
    eC = np.zeros((18, 2, 32), np.float32)
    eC[17, 0, 0:16] = 1.0
    eC[3, 0, 16] = 1.0
    c8["eselC"] = eC.reshape(18, 64)

    def tb(b, nm):
        return np.ascontiguousarray(f(b).reshape(nm, 128).T)

    rw = np.float32(np.asarray(inputs["residual_weight"]))
    cf = {
        "bb2t": tb(inputs["bb2"], 4),
        "bb3t": tb(inputs["bb3"], 2), "tb2t": tb(inputs["tb2"], 2),
        "c13": (rw * (f(inputs["pb"]) + f(inputs["qb2"]))).reshape(SD, 1),
        "rw13": np.full((SD, 1), rw, np.float32),
        "id13": np.eye(SD, dtype=np.float32),
    }
    c16 = {"sl21": sl21, "tw1b": tw1b, "qw1b": qw1b,
           "ones2n": np.ones((16, 2 * NB), np.float32)}

    blob8 = np.zeros((128, C8W), E4NP)
    for name, (o, p, w) in C8.items():
        blob8[0:p, o:o + w] = c8[name].astype(E4NP)
    blob16 = np.zeros((128, C16W), np.float16)
    for name, (o, p, w) in C16.items():
        blob16[0:p, o:o + w] = c16[name].astype(np.float16)
    blobf = np.zeros((128, CFW), np.float32)
    for name, (o, p, w) in CF.items():
        blobf[0:p, o:o + w] = cf[name]

    # stac16: fp16 features, padded to 128 cols for the XBAR DMA transpose
    st = f(inputs["state"]); ac = f(inputs["action"])
    B = st.shape[0]
    stac16 = np.zeros((B, 128), np.float16)
    stac16[:, 0:3] = st[:, 0:3]
    stac16[:, 3] = 1.0
    stac16[:, 4:14] = st[:, 3:13]
    stac16[:, 14:18] = ac
    stac16[:, 18:21] = np.square(st[:, 0:3])
    return dict(blob8=blob8, blob16=blob16, blobf=blobf), stac16


def _core_inputs(inputs, common=None):
    """Build the 8 per-core input maps from the full problem inputs."""
    if common is None:
        common, stac16 = _host_prep(inputs)
    else:
        common, stac16 = common
    state = np.ascontiguousarray(np.asarray(inputs["state"], np.float32))
    in_maps = []
    for i in range(N_CORES):
        m = dict(common)
        m["state"] = state[i * RPC:(i + 1) * RPC]
        m["stac16"] = stac16[i * RPC:(i + 1) * RPC]
        in_maps.append(m)
    return in_maps


_NC_CACHE = {}


def _get_nc(rpc=RPC):
    if rpc not in _NC_CACHE:
        _NC_CACHE[rpc] = build_nc(rpc)
    return _NC_CACHE[rpc]


def kernel(**inputs):
    from concourse.bass_utils import run_bass_kernel_spmd

    nc = _get_nc()
    in_maps = _core_inputs(inputs)
    res = run_bass_kernel_spmd(nc, in_maps, list(range(N_CORES)))
    return np.concatenate([r["out"] for r in res.results], axis=0)


# revision 19
# speedup vs baseline: 1.0150x; 1.0150x over previous
"""DeepONet-style neural operator forward pass on 8 TRN2 NeuronCores.

Pure data parallel over the batch (131072 rows -> 16384/core), weights
replicated. Activations live feature-major ([feat, rows]); rows are
processed in blocks of 512 (one fp32 PSUM bank).

This version runs the heavy layers in fp8e4m3 with DoubleRow matmuls
(0.5 PE cycles/row, 2x the fp16 rate): L1 544->1024, L2 1024->512,
L3 512->256, trunk 256->256 and the output heads. Layout: activations are
kept as "pair" tiles [128, 2, N] so a 256-deep contraction is one
DoubleRow matmul. Host supplies a fp16 feature matrix stac16 [rows, 128]
(pos, ones, state, action, pos^2 columns) that is DMA-transposed (XBAR)
straight into SBUF, eliminating all PE input transposes; sensor dist^2 is
one K=21 fp16 matmul (|s|^2 and |pos|^2 folded in via ones/pos^2 rows),
sqrt runs as fp16 magic-rsqrt + 1 Newton step on the DVE, and exp on ACT.
"""

import numpy as np

import concourse.bass as bass
import concourse.mybir as mybir
import concourse.tile as tile
from concourse import bacc

F32 = mybir.dt.float32
F16 = mybir.dt.float16
E4 = mybir.dt.float8e4
I16 = mybir.dt.int16
I32 = mybir.dt.int32
AF = mybir.ActivationFunctionType
ALU = mybir.AluOpType
AX = mybir.AxisListType
DR = mybir.MatmulPerfMode.DoubleRow

SD = 13          # state dim
AD = 4           # action dim
J = SD + AD      # 17 per-sensor features
NS = 32          # sensors
H1, H2, H4, H8 = 1024, 512, 256, 128
B_FULL = 131072
N_CORES = 8
RPC = B_FULL // N_CORES   # rows per core
NB = 512                  # rows per block (= fp32 PSUM bank)

# stacT row layout: 0-2 pos, 3 ones, 4-13 state[3:], 14-17 action, 18-20 pos^2
ROWMAP = [j if j < 3 else j + 1 for j in range(J)]   # feature j -> stacT row

# engine assignment for the relu after L1 (4 merged m-pairs) and L2 (4 m-tiles)
RELU1_ENG = ["act", "vector", "act", "vector", "act", "vector", "act", "vector"]
RELU2_ENG = ["vector", "act", "vector", "act"]


def _const_specs():
    b8 = [("w1A", 128, 2048), ("w1B", 128, 2048), ("w1C", 32, 2048)]
    for k in range(4):
        b8.append((f"w2_{k}", 128, 1024))
    for k in range(2):
        b8.append((f"w3_{k}", 128, 512))
    b8 += [("tw2p", 128, 512), ("pwp", 128, 32), ("qw2p", 128, 32)]
    for t in range(4):
        b8.append((f"esel_{t}", 18, 256))
    b8.append(("eselC", 18, 64))
    b16 = [("sl21", 21, 128), ("tw1b", 4, 256), ("qw1b", 4, 128),
           ("ones2n", 16, 2 * NB)]
    bf = [("bb2t", 128, 4), ("bb3t", 128, 2),
          ("tb2t", 128, 2), ("c13", SD, 1), ("rw13", SD, 1),
          ("id13", SD, SD)]

    def offsets(specs):
        out, o = {}, 0
        for name, p, w in specs:
            out[name] = (o, p, w)
            o += w
        return out, o
    o8, w8 = offsets(b8)
    o16, w16 = offsets(b16)
    of, wf = offsets(bf)
    return o8, w8, o16, w16, of, wf


C8, C8W, C16, C16W, CF, CFW = _const_specs()


def build_nc(rpc=RPC, repeats=1):
    assert rpc % NB == 0
    nblk = rpc // NB
    nc = bacc.Bacc(trn_type="TRN2")

    def inp(name, shape, dt=F32):
        return nc.dram_tensor(name, shape, dt, kind="ExternalInput").ap()

    state = inp("state", [rpc, SD])
    stac16 = inp("stac16", [rpc, 128], F16)
    blob8 = inp("blob8", [128, C8W], E4)
    blob16 = inp("blob16", [128, C16W], F16)
    blobf = inp("blobf", [128, CFW])

    out = nc.dram_tensor("out", [rpc, SD], F32, kind="ExternalOutput").ap()

    with tile.TileContext(nc) as tc:
        for _rep in range(repeats):
            _body(tc, nblk, locals())
    nc.compile()
    return nc


def _body(tc, nblk, t):
    nc = tc.nc

    import contextlib
    stack = contextlib.ExitStack()
    consts = stack.enter_context(tc.tile_pool(name="consts", bufs=1))
    sb_in = stack.enter_context(tc.tile_pool(name="sb_in", bufs=1))
    sb_act = stack.enter_context(tc.tile_pool(name="sb_act", bufs=1))
    sb_sm = stack.enter_context(tc.tile_pool(name="sb_sm", bufs=1))
    ps_mm = stack.enter_context(tc.tile_pool(name="ps_mm", bufs=4, space="PSUM"))
    ps_pair = stack.enter_context(tc.tile_pool(name="ps_pair", bufs=2, space="PSUM"))

    blob8_sb = consts.tile([128, C8W], E4, name="blob8_sb", tag="blob8_sb")
    blob16_sb = consts.tile([128, C16W], F16, name="blob16_sb", tag="blob16_sb")
    blobf_sb = consts.tile([128, CFW], F32, name="blobf_sb", tag="blobf_sb")
    NCH = 8
    step = (C8W + NCH - 1) // NCH
    for i in range(NCH):
        a, b = i * step, min((i + 1) * step, C8W)
        nc.sync.dma_start(out=blob8_sb[:, a:b], in_=t["blob8"][:, a:b])
    nc.sync.dma_start(out=blob16_sb, in_=t["blob16"])
    nc.sync.dma_start(out=blobf_sb, in_=t["blobf"])

    def v8(name):
        o, p, w = C8[name]
        return blob8_sb[0:p, o:o + w]

    def v16(name):
        o, p, w = C16[name]
        return blob16_sb[0:p, o:o + w]

    def vf(name):
        o, p, w = CF[name]
        return blobf_sb[0:p, o:o + w]

    def pair_view(name, m=None, mw=None):
        o, p, w = C8[name]
        r = blob8_sb[0:p, o:o + w].rearrange("p (i m) -> p i m", i=2)
        if m is None:
            return r
        return r[:, :, m * mw:(m + 1) * mw]

    w1A = [pair_view("w1A", m, 128) for m in range(8)]
    w1B = [pair_view("w1B", m, 128) for m in range(8)]
    w1C = [pair_view("w1C", m, 128) for m in range(8)]
    ones2n = v16("ones2n")
    w2p = [[pair_view(f"w2_{k}", m, 128) for m in range(4)] for k in range(4)]
    w3p = [[pair_view(f"w3_{k}", m, 128) for m in range(2)] for k in range(2)]
    tw2p = [pair_view("tw2p", m, 128) for m in range(2)]
    pwp = pair_view("pwp")
    qw2p = pair_view("qw2p")
    esel8 = [pair_view(f"esel_{t_}") for t_ in range(4)]
    eselC = pair_view("eselC")
    sl21 = v16("sl21")
    tw1b = v16("tw1b")
    qw1b = v16("qw1b")
    bb2sb = vf("bb2t")
    bb3sb = vf("bb3t")
    tb2sb = vf("tb2t")
    c13sb = vf("c13")
    rw13sb = vf("rw13")
    id13sb = vf("id13")
    zero1 = consts.tile([128, 1], F32)
    nc.vector.memset(zero1, 0.0)

    state, stac16, outdr = t["state"], t["stac16"], t["out"]

    def relu_op(eng, out, ps, bias_col):
        if eng == "act":
            nc.scalar.activation(out=out, in_=ps, func=AF.Relu,
                                 bias=bias_col, scale=1.0)
        else:
            e = nc.gpsimd if eng == "pool" else nc.vector
            e.tensor_scalar(out=out, in0=ps, scalar1=bias_col, scalar2=0.0,
                            op0=ALU.add, op1=ALU.max)

    ablk = {}

    def stage_a(blk):
        r0 = blk * NB
        st_ac = sb_in.tile([128, 4, SD], F32, tag="st_ac", bufs=3)
        nc.sync.dma_start(
            out=st_ac,
            in_=state[r0:r0 + NB, :].rearrange("(c p) d -> p c d", p=128))
        stacT = sb_in.tile([128, NB], F16, tag="stacT", bufs=3)
        nc.sync.dma_start(out=stacT, in_=stac16[r0:r0 + NB, :],
                          transpose=True)

        # q = dist^2 (K=21 fp16 matmul; |s|^2, |pos|^2 folded via const rows)
        q_ps = ps_mm.tile([128, NB], F32, tag="mm", bufs=4)
        nc.tensor.matmul(q_ps, sl21, stacT[0:21, :], start=True, stop=True)
        qs = sb_sm.tile([128, NB], F16, tag="qs", bufs=3)
        nc.scalar.activation(out=qs, in_=q_ps, func=AF.Relu,
                             bias=zero1[:, 0:1], scale=1.0)

        # dist = q * rsqrt(q): fp16 magic seed + 1 Newton (gpsimd, SBUF only)
        r = sb_sm.tile([128, NB], F16, tag="r", bufs=3)
        y = sb_sm.tile([128, NB], F16, tag="y", bufs=3)
        u = sb_sm.tile([128, NB], F16, tag="u", bufs=3)
        nc.vector.tensor_scalar(
            out=r.bitcast(I16), in0=qs.bitcast(I16), scalar1=1, scalar2=None,
            op0=ALU.logical_shift_right)
        nc.vector.tensor_scalar(
            out=r.bitcast(I16), in0=r.bitcast(I16), scalar1=-1,
            scalar2=0x59BA, op0=ALU.mult, op1=ALU.add)
        nc.gpsimd.tensor_mul(y, qs, r)
        nc.gpsimd.tensor_mul(u, y, r)
        nc.gpsimd.tensor_scalar(out=u, in0=u, scalar1=-0.5, scalar2=1.5,
                                op0=ALU.mult, op1=ALU.add)
        nc.gpsimd.tensor_mul(y, y, u)   # y = dist

        w16 = sb_in.tile([128, NB], F16, tag="w16", bufs=3)
        nc.scalar.activation(out=w16, in_=y, func=AF.Exp,
                             bias=zero1[:, 0:1], scale=-2.0)

        # wrepC[p, i, n] = w16[i*16+p, n]; rows 16-31 = ones (bias + pad)
        wrepC = sb_in.tile([32, 2, NB], F16, tag="wrepC", bufs=3)
        nc.sync.dma_start(out=wrepC[0:16, 0, :], in_=w16[0:16, :])
        nc.sync.dma_start(out=wrepC[0:16, 1, :], in_=w16[16:32, :])
        nc.sync.dma_start(out=wrepC[16:32, :, :],
                          in_=ones2n.rearrange("p (i n) -> p i n", i=2))

        stac8 = sb_in.tile([18, NB], E4, tag="stac8", bufs=3)
        nc.gpsimd.tensor_copy(stac8, stacT[0:18, :])
        s8b = stac8[:, None, :].broadcast_to([18, 2, NB])

        # replicate features to enc channels (DoubleRow esel matmuls), cast
        # psum->fp16 via gpsimd DGE DMA, multiply by sensor weights on
        # gpsimd -> fp8 enc pair tiles
        enc = []
        srepC = ps_pair.tile([128, 2, NB], F32, tag="pair", bufs=2)
        nc.tensor.matmul(srepC[0:32, 0, :], eselC, s8b, start=True,
                         stop=True, perf_mode=DR)
        nc.tensor.matmul(srepC[0:32, 1, :], eselC, s8b, start=True,
                         stop=True, perf_mode=DR)
        etC = sb_in.tile([32, 2, NB], E4, tag="encC", bufs=3)
        nc.vector.tensor_mul(etC, srepC[0:32, :, :], wrepC)
        for p_ in range(2):
            et = sb_in.tile([128, 2, NB], E4, tag=f"enc{p_}", bufs=3,
                            name=f"et{p_}")
            for i_ in range(2):
                srep = ps_mm.tile([128, NB], F32, tag="mm", bufs=4)
                nc.tensor.matmul(srep, esel8[2 * p_ + i_], s8b, start=True,
                                 stop=True, perf_mode=DR)
                nc.vector.tensor_mul(et[:, i_, :], srep, w16)
            enc.append(et)
        enc.append(etC)
        ablk[blk] = dict(st_ac=st_ac, stacT=stacT, enc=enc)

    def stage_b(blk):
        st = ablk[blk]
        enc, stacT = st["enc"], st["stacT"]

        # ---- branch L1: 544 -> 1024, bias pre-added via the ones channel;
        # two m-tiles share one [128, 2, NB] psum -> single merged relu ----
        h1 = [sb_act.tile([128, 2, NB], E4, tag=f"h1_{k}", bufs=2,
                          name=f"h1_{k}") for k in range(4)]
        for m in range(8):
            ps = ps_mm.tile([128, NB], F32, tag="mm", bufs=4)
            nc.tensor.matmul(ps, w1A[m], enc[0], start=True,
                             stop=False, perf_mode=DR)
            nc.tensor.matmul(ps, w1B[m], enc[1], start=False,
                             stop=False, perf_mode=DR)
            nc.tensor.matmul(ps, w1C[m], enc[2], start=False,
                             stop=True, perf_mode=DR)
            eng = RELU1_ENG[m]
            dst = h1[m // 2][:, m % 2, :]
            if eng == "act":
                nc.scalar.activation(out=dst, in_=ps, func=AF.Relu,
                                     bias=zero1[:, 0:1], scale=1.0)
            else:
                nc.vector.tensor_scalar_max(dst, ps, 0.0)

        # ---- branch L2: 1024 -> 512 (4 DoubleRow matmuls per m-tile) ----
        h2 = [sb_act.tile([128, 2, NB], E4, tag=f"h2_{k}", bufs=2,
                          name=f"h2_{k}") for k in range(2)]
        for m in range(4):
            ps = ps_mm.tile([128, NB], F32, tag="mm", bufs=4)
            for k in range(4):
                nc.tensor.matmul(ps, w2p[k][m], h1[k], start=(k == 0),
                                 stop=(k == 3), perf_mode=DR)
            relu_op(RELU2_ENG[m], h2[m // 2][:, m % 2, :], ps,
                    bb2sb[:, m:m + 1])

        # ---- trunk: tanh(pos@tw1+tb1) [bias folded], tanh(.@tw2+tb2) ----
        tt_ps = ps_pair.tile([128, 2, NB], F32, tag="pair", bufs=2)
        nc.tensor.matmul(tt_ps[:, 0, :], tw1b[:, 0:128], stacT[0:4, :],
                         start=True, stop=True)
        nc.tensor.matmul(tt_ps[:, 1, :], tw1b[:, 128:256], stacT[0:4, :],
                         start=True, stop=True)
        tt8 = sb_act.tile([128, 2, NB], E4, tag="tt8", bufs=2)
        nc.scalar.activation(out=tt8, in_=tt_ps, func=AF.Tanh,
                             bias=zero1[:, 0:1], scale=1.0)
        trunk = []
        for m in range(2):
            ps = ps_mm.tile([128, NB], F32, tag="mm", bufs=4)
            nc.tensor.matmul(ps, tw2p[m], tt8, start=True, stop=True,
                             perf_mode=DR)
            tm = sb_act.tile([128, NB], F16, tag="trunk", bufs=3)
            nc.scalar.activation(out=tm, in_=ps, func=AF.Tanh,
                                 bias=tb2sb[:, m:m + 1], scale=1.0)
            trunk.append(tm)

        # ---- qnet hidden: relu(pos@qw1+qb1) [bias folded] ----
        ps = ps_mm.tile([128, NB], F32, tag="mm", bufs=4)
        nc.tensor.matmul(ps, qw1b, stacT[0:4, :], start=True, stop=True)
        bq8 = sb_act.tile([128, NB], E4, tag="bq8", bufs=2)
        nc.scalar.activation(out=bq8, in_=ps, func=AF.Relu,
                             bias=zero1[:, 0:1], scale=1.0)

        # ---- branch L3 (+bias) fused with interaction multiply ----
        inter = sb_act.tile([128, 2, NB], E4, tag="inter", bufs=2)
        for m in range(2):
            ps = ps_mm.tile([128, NB], F32, tag="mm", bufs=4)
            for k in range(2):
                nc.tensor.matmul(ps, w3p[k][m], h2[k], start=(k == 0),
                                 stop=(k == 1), perf_mode=DR)
            nc.vector.scalar_tensor_tensor(
                out=inter[:, m, :], in0=ps, scalar=bb3sb[:, m:m + 1],
                in1=trunk[m], op0=ALU.add, op1=ALU.mult)

        # ---- tail: (inter@pw + bq@qw2) in one psum, DoubleRow ----
        tail_full = ps_mm.tile([128, NB], F32, tag="mm", bufs=4)
        tail_ps = tail_full[0:16, :]
        nc.tensor.matmul(tail_ps, pwp, inter, start=True, stop=False,
                         perf_mode=DR)
        nc.tensor.matmul(tail_ps, qw2p,
                         bq8[:, None, :].broadcast_to([128, 2, NB]),
                         start=False, stop=True, perf_mode=DR)
        combT = sb_sm.tile([SD, NB], F32, tag="combT", bufs=2)
        nc.vector.tensor_scalar(
            out=combT, in0=tail_full[0:SD, :], scalar1=rw13sb[:, 0:1],
            scalar2=c13sb[:, 0:1], op0=ALU.mult, op1=ALU.add)
        ablk[blk]["combT"] = combT

    def stage_c(blk):
        r0 = blk * NB
        st = ablk.pop(blk)
        st_ac, combT = st["st_ac"], st["combT"]
        trt = ps_mm.tile([128, NB], F32, tag="mm", bufs=4)
        nxt = sb_sm.tile([128, 4, SD], F32, tag="nxt", bufs=2)
        sq = sb_sm.tile([128, 4, 4], F32, tag="sq", bufs=2)
        for c in range(4):
            tr_ps = trt[:, 16 * c:16 * c + SD]
            nc.tensor.transpose(tr_ps, combT[:, c * 128:(c + 1) * 128],
                                id13sb)
            nc.vector.tensor_add(nxt[:, c, :], tr_ps, st_ac[:, c, :])
            nc.vector.tensor_mul(sq[:, c, :], nxt[:, c, 3:7], nxt[:, c, 3:7])
        qn = sb_sm.tile([128, 4], F32, tag="qn", bufs=2)
        nc.vector.reduce_sum(out=qn.rearrange("p (c o) -> p c o", o=1),
                             in_=sq, axis=AX.X)
        rq = sb_sm.tile([128, 4], F32, tag="rq", bufs=2)
        uq = sb_sm.tile([128, 4], F32, tag="uq", bufs=2)
        yq = sb_sm.tile([128, 4], F32, tag="yq", bufs=2)
        nc.vector.tensor_scalar(
            out=rq.bitcast(I32), in0=qn.bitcast(I32), scalar1=1, scalar2=None,
            op0=ALU.arith_shift_right)
        nc.vector.tensor_scalar(
            out=rq.bitcast(I32), in0=rq.bitcast(I32), scalar1=-1,
            scalar2=0x5F3759DF, op0=ALU.mult, op1=ALU.add)
        for it in range(2):
            nc.gpsimd.tensor_mul(yq, qn, rq)
            nc.gpsimd.tensor_mul(uq, yq, rq)
            nc.gpsimd.tensor_scalar(out=uq, in0=uq, scalar1=-0.5, scalar2=1.5,
                                    op0=ALU.mult, op1=ALU.add)
            nc.gpsimd.tensor_mul(rq, rq, uq)
        outt = sb_sm.tile([128, 4, SD], F32, tag="outt", bufs=2)
        nc.gpsimd.tensor_copy(outt, nxt)
        for c in range(4):
            nc.gpsimd.tensor_scalar_mul(
                outt[:, c, 3:7], nxt[:, c, 3:7], rq[:, c:c + 1])
        out_dst = outdr[r0:r0 + NB, :].rearrange("(c p) d -> p c d", p=128)
        nc.sync.dma_start(out=out_dst, in_=outt)

    # software-pipelined emission: A two blocks ahead of B/C
    stage_a(0)
    if nblk > 1:
        stage_a(1)
    for blk in range(nblk):
        stage_b(blk)
        stage_c(blk)
        if blk + 2 < nblk:
            stage_a(blk + 2)
    stack.close()


def _host_prep(inputs):
    """Precompute fp8/fp16 weight blobs and the padded feature matrix."""
    import ml_dtypes
    E4NP = ml_dtypes.float8_e4m3
    f = lambda x: np.ascontiguousarray(np.asarray(x, dtype=np.float32))
    sl = f(inputs["sensor_locations"])            # [32, 3]
    pidx = np.arange(128) % NS

    sl21 = np.zeros((21, 128), np.float32)
    sl21[0:3, :] = -2.0 * sl[pidx].T
    sl21[3, :] = np.square(sl).sum(1)[pidx]
    sl21[18:21, :] = 1.0
    tw1b = np.concatenate([f(inputs["tw1"]), f(inputs["tb1"])[None, :]], 0)
    qw1b = np.concatenate([f(inputs["qw1"]), f(inputs["qb1"])[None, :]], 0)

    # enc channel ch = j*32 + s  <-  original bw1 row s*17 + j
    ch = np.arange(544)
    w1p = f(inputs["bw1"])[(ch % NS) * J + ch // NS, :]        # [544, 1024]

    def drpack(w):           # [2*P, M] -> [P, 2, M] -> [P, 2*M]
        p2 = w.shape[0] // 2
        return w.reshape(2, p2, -1).transpose(1, 0, 2).reshape(p2, -1)

    # w1C rows: 16 feature channels + bias channel (bb1 | 0) + zero pad
    w1c = np.zeros((32, 2, H1), np.float32)
    w1c[0:16] = w1p[512:544].reshape(2, 16, H1).transpose(1, 0, 2)
    w1c[16, 0, :] = f(inputs["bb1"])
    c8 = {
        "w1A": drpack(w1p[0:256]), "w1B": drpack(w1p[256:512]),
        "w1C": w1c.reshape(32, 2 * H1),
        "tw2p": drpack(f(inputs["tw2"])),
    }
    w2 = f(inputs["bw2"]); w3 = f(inputs["bw3"])
    for k in range(4):
        c8[f"w2_{k}"] = drpack(w2[k * 256:(k + 1) * 256])
    for k in range(2):
        c8[f"w3_{k}"] = drpack(w3[k * 256:(k + 1) * 256])
    pw = f(inputs["pw"])                                       # [256, 13]
    pwp = drpack(pw)                                           # [128, 26]
    c8["pwp"] = np.concatenate(
        [pwp.reshape(128, 2, SD),
         np.zeros((128, 2, 16 - SD), np.float32)], -1).reshape(128, 32)
    qw2 = f(inputs["qw2"])                                     # [128, 13]
    qw2p = np.stack([qw2, np.zeros_like(qw2)], 1)              # [128, 2, 13]
    c8["qw2p"] = np.concatenate(
        [qw2p, np.zeros((128, 2, 16 - SD), np.float32)], -1).reshape(128, 32)
    for t in range(4):
        e = np.zeros((18, 2, 128), np.float32)
        for m in range(128):
            e[ROWMAP[(128 * t + m) // NS], 0, m] = 1.0
        c8[f"esel_{t}"] = e.reshape(18, 256)
    # eselC: cols 0-15 select the action[3] feature row (17); col 16 selects
    # the ones row (3) -> bias channel for L1
    eC = np.zeros((18, 2, 32), np.float32)
    eC[17, 0, 0:16] = 1.0
    eC[3, 0, 16] = 1.0
    c8["eselC"] = eC.reshape(18, 64)

    def tb(b, nm):
        return np.ascontiguousarray(f(b).reshape(nm, 128).T)

    rw = np.float32(np.asarray(inputs["residual_weight"]))
    cf = {
        "bb2t": tb(inputs["bb2"], 4),
        "bb3t": tb(inputs["bb3"], 2), "tb2t": tb(inputs["tb2"], 2),
        "c13": (rw * (f(inputs["pb"]) + f(inputs["qb2"]))).reshape(SD, 1),
        "rw13": np.full((SD, 1), rw, np.float32),
        "id13": np.eye(SD, dtype=np.float32),
    }
    c16 = {"sl21": sl21, "tw1b": tw1b, "qw1b": qw1b,
           "ones2n": np.ones((16, 2 * NB), np.float32)}

    blob8 = np.zeros((128, C8W), E4NP)
    for name, (o, p, w) in C8.items():
        blob8[0:p, o:o + w] = c8[name].astype(E4NP)
    blob16 = np.zeros((128, C16W), np.float16)
    for name, (o, p, w) in C16.items():
        blob16[0:p, o:o + w] = c16[name].astype(np.float16)
    blobf = np.zeros((128, CFW), np.float32)
    for name, (o, p, w) in CF.items():
        blobf[0:p, o:o + w] = cf[name]

    # stac16: fp16 features, padded to 128 cols for the XBAR DMA transpose
    st = f(inputs["state"]); ac = f(inputs["action"])
    B = st.shape[0]
    stac16 = np.zeros((B, 128), np.float16)
    stac16[:, 0:3] = st[:, 0:3]
    stac16[:, 3] = 1.0
    stac16[:, 4:14] = st[:, 3:13]
    stac16[:, 14:18] = ac
    stac16[:, 18:21] = np.square(st[:, 0:3])
    return dict(blob8=blob8, blob16=blob16, blobf=blobf), stac16


def _core_inputs(inputs, common=None):
    """Build the 8 per-core input maps from the full problem inputs."""
    if common is None:
        common, stac16 = _host_prep(inputs)
    else:
        common, stac16 = common
    state = np.ascontiguousarray(np.asarray(inputs["state"], np.float32))
    in_maps = []
    for i in range(N_CORES):
        m = dict(common)
        m["state"] = state[i * RPC:(i + 1) * RPC]
        m["stac16"] = stac16[i * RPC:(i + 1) * RPC]
        in_maps.append(m)
    return in_maps


_NC_CACHE = {}


def _get_nc(rpc=RPC):
    if rpc not in _NC_CACHE:
        _NC_CACHE[rpc] = build_nc(rpc)
    return _NC_CACHE[rpc]


def kernel(**inputs):
    from concourse.bass_utils import run_bass_kernel_spmd

    nc = _get_nc()
    in_maps = _core_inputs(inputs)
    res = run_bass_kernel_spmd(nc, in_maps, list(range(N_CORES)))
    return np.concatenate([r["out"] for r in res.results], axis=0)


# revision 20
# speedup vs baseline: 1.1980x; 1.1804x over previous
"""DeepONet-style neural operator forward pass on 8 TRN2 NeuronCores.

Pure data parallel over the batch (131072 rows -> 16384/core), weights
replicated, activations feature-major ([feat, rows]), 512-row blocks.

Key layout trick: the host emits a single fp16 matrix stac16w [rows, 768]
whose columns are (a) the 21 feature/aux values (pos, ones, state, action,
pos^2) and (b) all 544 sensor-replicated enc channels (j-major) plus an
ones bias channel. Each block issues 6 XBAR DMA-transposes (one per
128-column chunk) straight into SBUF, so the PE never transposes inputs
and never runs replication matmuls. Sensor dist^2 is one K=21 fp16 matmul
(|s|^2, |pos|^2 folded via ones/pos^2 rows), sqrt is a fp16 magic-rsqrt +
1 Newton step, exp on ACT, and the enc = srep * w multiplies run on the
otherwise-idle GPSIMD engine (SBUF-only). All matmuls are fp16 (1 col/cyc;
fp8 DoubleRow is power-throttled to a 50% duty cycle on this part, so it
buys nothing sustained). L1 bias is folded via the ones channel; tt/qnet
biases via ones rows; bb2/bb3/tb2 via per-partition bias pointers.
"""

import numpy as np

import concourse.bass as bass
import concourse.mybir as mybir
import concourse.tile as tile
from concourse import bacc

F32 = mybir.dt.float32
F16 = mybir.dt.float16
I16 = mybir.dt.int16
I32 = mybir.dt.int32
AF = mybir.ActivationFunctionType
ALU = mybir.AluOpType
AX = mybir.AxisListType

SD = 13          # state dim
AD = 4           # action dim
J = SD + AD      # 17 per-sensor features
NS = 32          # sensors
H1, H2, H4, H8 = 1024, 512, 256, 128
B_FULL = 131072
N_CORES = 8
RPC = B_FULL // N_CORES   # rows per core
NB = 512                  # rows per block (= fp32 PSUM bank)
WC = 768                  # stac16w columns (6 chunks of 128)

# stacT row layout (chunk 0): 0-2 pos, 3 ones, 4-13 state[3:], 14-17 action,
# 18-20 pos^2.  Chunks 1-4: enc channels 0..511 (ch = j*32+s).  Chunk 5:
# rows 0-31 = enc channels 512..543, row 32 = ones (L1 bias channel).
ROWMAP = [j if j < 3 else j + 1 for j in range(J)]

# engine split for the relus after L1 (8 m-tiles) and L2 (4 m-tiles)
RELU1_ENG = ["act", "vector", "act", "vector", "act", "vector", "act",
             "vector"]
RELU2_ENG = ["vector", "act", "vector", "act"]


def _const_specs():
    b16 = []
    for k in range(4):
        b16.append((f"w1_{k}", 128, H1))
    b16.append(("w1_4", 33, H1))          # 32 enc channels + bias row
    for k in range(8):
        b16.append((f"w2_{k}", 128, H2))
    for k in range(4):
        b16.append((f"w3_{k}", 128, H4))
    for k in range(2):
        b16.append((f"tw2_{k}", 128, H4))
    b16 += [("pw_0", 128, SD), ("pw_1", 128, SD), ("qw2", H8, SD),
            ("sl21", 21, 128), ("tw1b", 4, 256), ("qw1b", 4, 128)]
    bf = [("bb2t", 128, 4), ("bb3t", 128, 2), ("tb2t", 128, 2),
          ("c13", SD, 1), ("rw13", SD, 1), ("id13", SD, SD)]

    def offsets(specs):
        out, o = {}, 0
        for name, p, w in specs:
            out[name] = (o, p, w)
            o += w
        return out, o
    o16, w16 = offsets(b16)
    of, wf = offsets(bf)
    return o16, w16, of, wf


C16, C16W, CF, CFW = _const_specs()


def build_nc(rpc=RPC, repeats=1):
    assert rpc % NB == 0
    nblk = rpc // NB
    nc = bacc.Bacc(trn_type="TRN2")

    def inp(name, shape, dt=F32):
        return nc.dram_tensor(name, shape, dt, kind="ExternalInput").ap()

    state = inp("state", [rpc, SD])
    stac16w = inp("stac16w", [rpc, WC], F16)
    blob16 = inp("blob16", [128, C16W], F16)
    blobf = inp("blobf", [128, CFW])

    out = nc.dram_tensor("out", [rpc, SD], F32, kind="ExternalOutput").ap()

    with tile.TileContext(nc) as tc:
        for _rep in range(repeats):
            _body(tc, nblk, locals())
    nc.compile()
    return nc


def _body(tc, nblk, t):
    nc = tc.nc

    import contextlib
    stack = contextlib.ExitStack()
    consts = stack.enter_context(tc.tile_pool(name="consts", bufs=1))
    sb_in = stack.enter_context(tc.tile_pool(name="sb_in", bufs=1))
    sb_act = stack.enter_context(tc.tile_pool(name="sb_act", bufs=1))
    sb_sm = stack.enter_context(tc.tile_pool(name="sb_sm", bufs=1))
    ps_mm = stack.enter_context(tc.tile_pool(name="ps_mm", bufs=6,
                                             space="PSUM"))
    ps_pair = stack.enter_context(tc.tile_pool(name="ps_pair", bufs=1,
                                               space="PSUM"))

    blob16_sb = consts.tile([128, C16W], F16, name="blob16_sb",
                            tag="blob16_sb")
    blobf_sb = consts.tile([128, CFW], F32, name="blobf_sb", tag="blobf_sb")
    NCH = 8
    step = (C16W + NCH - 1) // NCH
    for i in range(NCH):
        a, b = i * step, min((i + 1) * step, C16W)
        nc.sync.dma_start(out=blob16_sb[:, a:b], in_=t["blob16"][:, a:b])
    nc.sync.dma_start(out=blobf_sb, in_=t["blobf"])

    def v16(name):
        o, p, w = C16[name]
        return blob16_sb[0:p, o:o + w]

    def vf(name):
        o, p, w = CF[name]
        return blobf_sb[0:p, o:o + w]

    w1sb = [v16(f"w1_{k}") for k in range(5)]
    w2sb = [v16(f"w2_{k}") for k in range(8)]
    w3sb = [v16(f"w3_{k}") for k in range(4)]
    tw2sb = [v16(f"tw2_{k}") for k in range(2)]
    pwsb = [v16("pw_0"), v16("pw_1")]
    qw2sb = v16("qw2")
    sl21 = v16("sl21")
    tw1b = v16("tw1b")
    qw1b = v16("qw1b")
    bb2sb = vf("bb2t")
    bb3sb = vf("bb3t")
    tb2sb = vf("tb2t")
    c13sb = vf("c13")
    rw13sb = vf("rw13")
    id13sb = vf("id13")
    zero1 = consts.tile([128, 1], F32)
    nc.vector.memset(zero1, 0.0)

    state, stac16w, outdr = t["state"], t["stac16w"], t["out"]

    ablk = {}

    def stage_a(blk):
        r0 = blk * NB
        st_ac = sb_in.tile([128, 4, SD], F32, tag="st_ac", bufs=3)
        nc.sync.dma_start(
            out=st_ac,
            in_=state[r0:r0 + NB, :].rearrange("(c p) d -> p c d", p=128))
        # 6 XBAR transposes: chunk 0 = features, 1-5 = enc channel groups
        stacT = sb_in.tile([128, NB], F16, tag="stacT", bufs=3)
        nc.sync.dma_start(out=stacT, in_=stac16w[r0:r0 + NB, 0:128],
                          transpose=True)
        srep = []
        for c in range(5):
            s_ = sb_in.tile([128, NB], F16, tag=f"srep{c}", bufs=3,
                            name=f"srep{c}")
            eng = nc.scalar if c % 2 else nc.sync
            eng.dma_start(
                out=s_,
                in_=stac16w[r0:r0 + NB, 128 * (c + 1):128 * (c + 2)],
                transpose=True)
            srep.append(s_)

        # q = dist^2 (K=21 fp16 matmul; |s|^2, |pos|^2 folded via const rows)
        q_ps = ps_mm.tile([128, NB], F32, tag="mm", bufs=6)
        nc.tensor.matmul(q_ps, sl21, stacT[0:21, :], start=True, stop=True)
        qs = sb_sm.tile([128, NB], F16, tag="qs", bufs=3)
        nc.scalar.activation(out=qs, in_=q_ps, func=AF.Relu,
                             bias=zero1[:, 0:1], scale=1.0)

        # dist = q * rsqrt(q): fp16 magic seed + 1 Newton step
        r = sb_sm.tile([128, NB], F16, tag="r", bufs=3)
        y = sb_sm.tile([128, NB], F16, tag="y", bufs=3)
        u = sb_sm.tile([128, NB], F16, tag="u", bufs=3)
        nc.vector.tensor_scalar(
            out=r.bitcast(I16), in0=qs.bitcast(I16), scalar1=1, scalar2=None,
            op0=ALU.logical_shift_right)
        nc.vector.tensor_scalar(
            out=r.bitcast(I16), in0=r.bitcast(I16), scalar1=-1,
            scalar2=0x59BA, op0=ALU.mult, op1=ALU.add)
        nc.gpsimd.tensor_mul(y, qs, r)
        nc.gpsimd.tensor_mul(u, y, r)
        nc.gpsimd.tensor_scalar(out=u, in0=u, scalar1=-0.5, scalar2=1.5,
                                op0=ALU.mult, op1=ALU.add)
        nc.gpsimd.tensor_mul(y, y, u)   # y = dist

        w16 = sb_in.tile([128, NB], F16, tag="w16", bufs=3)
        nc.scalar.activation(out=w16, in_=y, func=AF.Exp,
                             bias=zero1[:, 0:1], scale=-2.0)

        # enc = srep * w[p%32] on gpsimd (SBUF only); chunk 5: rows 0-31
        # are channels (*w), row 32 is the ones bias channel (copied).
        enc = []
        for c in range(4):
            et = sb_in.tile([128, NB], F16, tag=f"enc{c}", bufs=3,
                            name=f"enc{c}")
            nc.gpsimd.tensor_mul(et, srep[c], w16)
            enc.append(et)
        etC = sb_in.tile([33, NB], F16, tag="encC", bufs=3)
        nc.gpsimd.tensor_mul(etC[0:32, :], srep[4][0:32, :], w16[0:32, :])
        nc.vector.tensor_copy(etC[32:33, :], srep[4][32:33, :])
        enc.append(etC)
        ablk[blk] = dict(st_ac=st_ac, stacT=stacT, enc=enc)

    def stage_b(blk):
        st = ablk[blk]
        enc, stacT = st["enc"], st["stacT"]

        def relu_to(eng, dst, ps, bias_col=None):
            if eng == "act":
                nc.scalar.activation(
                    out=dst, in_=ps, func=AF.Relu,
                    bias=zero1[:, 0:1] if bias_col is None else bias_col,
                    scale=1.0)
            elif bias_col is None:
                nc.vector.tensor_scalar_max(dst, ps, 0.0)
            else:
                nc.vector.tensor_scalar(out=dst, in0=ps, scalar1=bias_col,
                                        scalar2=0.0, op0=ALU.add,
                                        op1=ALU.max)

        # ---- branch L1: 544 -> 1024 (5 fp16 chunks; bias pre-folded) ----
        h1 = [sb_act.tile([128, NB], F16, tag=f"h1_{m}", bufs=2,
                          name=f"h1_{m}") for m in range(8)]
        for m in range(8):
            ps = ps_mm.tile([128, NB], F32, tag="mm", bufs=6)
            for k in range(5):
                nc.tensor.matmul(ps, w1sb[k][:, m * 128:(m + 1) * 128],
                                 enc[k], start=(k == 0), stop=(k == 4))
            relu_to(RELU1_ENG[m], h1[m], ps)

        # ---- branch L2: 1024 -> 512 ----
        h2 = [sb_act.tile([128, NB], F16, tag=f"h2_{m}", bufs=2,
                          name=f"h2_{m}") for m in range(4)]
        for m in range(4):
            ps = ps_mm.tile([128, NB], F32, tag="mm", bufs=6)
            for k in range(8):
                nc.tensor.matmul(ps, w2sb[k][:, m * 128:(m + 1) * 128],
                                 h1[k], start=(k == 0), stop=(k == 7))
            relu_to(RELU2_ENG[m], h2[m], ps, bb2sb[:, m:m + 1])

        # ---- trunk: tanh(pos@tw1+tb1) [bias folded], tanh(.@tw2+tb2) ----
        tt_ps = ps_pair.tile([128, 2, NB], F32, tag="pair", bufs=1)
        nc.tensor.matmul(tt_ps[:, 0, :], tw1b[:, 0:128], stacT[0:4, :],
                         start=True, stop=True)
        nc.tensor.matmul(tt_ps[:, 1, :], tw1b[:, 128:256], stacT[0:4, :],
                         start=True, stop=True)
        tt = sb_act.tile([128, 2, NB], F16, tag="tt", bufs=2)
        nc.scalar.activation(out=tt, in_=tt_ps, func=AF.Tanh,
                             bias=zero1[:, 0:1], scale=1.0)
        trunk = []
        for m in range(2):
            ps = ps_mm.tile([128, NB], F32, tag="mm", bufs=6)
            for k in range(2):
                nc.tensor.matmul(ps, tw2sb[k][:, m * 128:(m + 1) * 128],
                                 tt[:, k, :], start=(k == 0), stop=(k == 1))
            tm = sb_act.tile([128, NB], F16, tag="trunk", bufs=3)
            nc.scalar.activation(out=tm, in_=ps, func=AF.Tanh,
                                 bias=tb2sb[:, m:m + 1], scale=1.0)
            trunk.append(tm)

        # ---- qnet hidden: relu(pos@qw1+qb1) [bias folded] ----
        ps = ps_mm.tile([128, NB], F32, tag="mm", bufs=6)
        nc.tensor.matmul(ps, qw1b, stacT[0:4, :], start=True, stop=True)
        bq = sb_act.tile([128, NB], F16, tag="bq", bufs=2)
        nc.scalar.activation(out=bq, in_=ps, func=AF.Relu,
                             bias=zero1[:, 0:1], scale=1.0)

        # ---- branch L3 (+bias) fused with interaction multiply ----
        inter = []
        for m in range(2):
            ps = ps_mm.tile([128, NB], F32, tag="mm", bufs=6)
            for k in range(4):
                nc.tensor.matmul(ps, w3sb[k][:, m * 128:(m + 1) * 128],
                                 h2[k], start=(k == 0), stop=(k == 3))
            im = sb_act.tile([128, NB], F16, tag=f"inter{m}", bufs=2,
                             name=f"inter{m}")
            nc.vector.scalar_tensor_tensor(
                out=im, in0=ps, scalar=bb3sb[:, m:m + 1], in1=trunk[m],
                op0=ALU.add, op1=ALU.mult)
            inter.append(im)

        # ---- tail: delta^T + bias_out^T accumulated in one psum ----
        tail_full = ps_mm.tile([128, NB], F32, tag="mm", bufs=6)
        tail_ps = tail_full[0:SD, :]
        nc.tensor.matmul(tail_ps, pwsb[0], inter[0], start=True, stop=False)
        nc.tensor.matmul(tail_ps, pwsb[1], inter[1], start=False, stop=False)
        nc.tensor.matmul(tail_ps, qw2sb, bq, start=False, stop=True)
        combT = sb_sm.tile([SD, NB], F32, tag="combT", bufs=2)
        nc.vector.tensor_scalar(
            out=combT, in0=tail_ps, scalar1=rw13sb[:, 0:1],
            scalar2=c13sb[:, 0:1], op0=ALU.mult, op1=ALU.add)
        ablk[blk]["combT"] = combT

    def stage_c(blk):
        r0 = blk * NB
        st = ablk.pop(blk)
        st_ac, combT = st["st_ac"], st["combT"]
        trt = ps_mm.tile([128, NB], F32, tag="mm", bufs=6)
        nxt = sb_sm.tile([128, 4, SD], F32, tag="nxt", bufs=2)
        sq = sb_sm.tile([128, 4, 4], F32, tag="sq", bufs=2)
        for c in range(4):
            tr_ps = trt[:, 16 * c:16 * c + SD]
            nc.tensor.transpose(tr_ps, combT[:, c * 128:(c + 1) * 128],
                                id13sb)
            nc.vector.tensor_add(nxt[:, c, :], tr_ps, st_ac[:, c, :])
            nc.vector.tensor_mul(sq[:, c, :], nxt[:, c, 3:7], nxt[:, c, 3:7])
        qn = sb_sm.tile([128, 4], F32, tag="qn", bufs=2)
        nc.vector.reduce_sum(out=qn.rearrange("p (c o) -> p c o", o=1),
                             in_=sq, axis=AX.X)
        rq = sb_sm.tile([128, 4], F32, tag="rq", bufs=2)
        uq = sb_sm.tile([128, 4], F32, tag="uq", bufs=2)
        yq = sb_sm.tile([128, 4], F32, tag="yq", bufs=2)
        nc.vector.tensor_scalar(
            out=rq.bitcast(I32), in0=qn.bitcast(I32), scalar1=1, scalar2=None,
            op0=ALU.arith_shift_right)
        nc.vector.tensor_scalar(
            out=rq.bitcast(I32), in0=rq.bitcast(I32), scalar1=-1,
            scalar2=0x5F3759DF, op0=ALU.mult, op1=ALU.add)
        for it in range(2):
            nc.gpsimd.tensor_mul(yq, qn, rq)
            nc.gpsimd.tensor_mul(uq, yq, rq)
            nc.gpsimd.tensor_scalar(out=uq, in0=uq, scalar1=-0.5, scalar2=1.5,
                                    op0=ALU.mult, op1=ALU.add)
            nc.gpsimd.tensor_mul(rq, rq, uq)
        outt = sb_sm.tile([128, 4, SD], F32, tag="outt", bufs=2)
        nc.gpsimd.tensor_copy(outt, nxt)
        for c in range(4):
            nc.gpsimd.tensor_scalar_mul(
                outt[:, c, 3:7], nxt[:, c, 3:7], rq[:, c:c + 1])
        out_dst = outdr[r0:r0 + NB, :].rearrange("(c p) d -> p c d", p=128)
        nc.sync.dma_start(out=out_dst, in_=outt)

    # software-pipelined emission: A two blocks ahead of B/C
    stage_a(0)
    if nblk > 1:
        stage_a(1)
    for blk in range(nblk):
        stage_b(blk)
        stage_c(blk)
        if blk + 2 < nblk:
            stage_a(blk + 2)
    stack.close()


def _host_prep(inputs):
    """Precompute fp16 weight blob and the wide replicated feature matrix."""
    f = lambda x: np.ascontiguousarray(np.asarray(x, dtype=np.float32))
    sl = f(inputs["sensor_locations"])            # [32, 3]
    pidx = np.arange(128) % NS

    sl21 = np.zeros((21, 128), np.float32)
    sl21[0:3, :] = -2.0 * sl[pidx].T
    sl21[3, :] = np.square(sl).sum(1)[pidx]
    sl21[18:21, :] = 1.0
    tw1b = np.concatenate([f(inputs["tw1"]), f(inputs["tb1"])[None, :]], 0)
    qw1b = np.concatenate([f(inputs["qw1"]), f(inputs["qb1"])[None, :]], 0)

    # enc channel ch = j*32 + s  <-  original bw1 row s*17 + j
    ch = np.arange(544)
    w1p = f(inputs["bw1"])[(ch % NS) * J + ch // NS, :]        # [544, 1024]

    c16 = {"sl21": sl21, "tw1b": tw1b, "qw1b": qw1b}
    for k in range(4):
        c16[f"w1_{k}"] = w1p[k * 128:(k + 1) * 128]
    w1c = np.zeros((33, H1), np.float32)
    w1c[0:32] = w1p[512:544]
    w1c[32] = f(inputs["bb1"])
    c16["w1_4"] = w1c
    w2 = f(inputs["bw2"]); w3 = f(inputs["bw3"]); tw2 = f(inputs["tw2"])
    for k in range(8):
        c16[f"w2_{k}"] = w2[k * 128:(k + 1) * 128]
    for k in range(4):
        c16[f"w3_{k}"] = w3[k * 128:(k + 1) * 128]
    for k in range(2):
        c16[f"tw2_{k}"] = tw2[k * 128:(k + 1) * 128]
    pw = f(inputs["pw"])
    c16["pw_0"] = pw[0:128]
    c16["pw_1"] = pw[128:256]
    c16["qw2"] = f(inputs["qw2"])

    def tb(b, nm):
        return np.ascontiguousarray(f(b).reshape(nm, 128).T)

    rw = np.float32(np.asarray(inputs["residual_weight"]))
    cf = {
        "bb2t": tb(inputs["bb2"], 4), "bb3t": tb(inputs["bb3"], 2),
        "tb2t": tb(inputs["tb2"], 2),
        "c13": (rw * (f(inputs["pb"]) + f(inputs["qb2"]))).reshape(SD, 1),
        "rw13": np.full((SD, 1), rw, np.float32),
        "id13": np.eye(SD, dtype=np.float32),
    }

    blob16 = np.zeros((128, C16W), np.float16)
    for name, (o, p, w) in C16.items():
        blob16[0:p, o:o + w] = c16[name].astype(np.float16)
    blobf = np.zeros((128, CFW), np.float32)
    for name, (o, p, w) in CF.items():
        blobf[0:p, o:o + w] = cf[name]

    # stac16w: fp16 features + host-replicated enc channels
    st = f(inputs["state"]); ac = f(inputs["action"])
    B = st.shape[0]
    feat = np.zeros((B, 21), np.float32)
    feat[:, 0:3] = st[:, 0:3]
    feat[:, 3] = 1.0
    feat[:, 4:14] = st[:, 3:13]
    feat[:, 14:18] = ac
    feat[:, 18:21] = np.square(st[:, 0:3])
    stac16w = np.zeros((B, WC), np.float16)
    stac16w[:, 0:21] = feat
    # channels ch = j*32+s -> column 128+ch; value = feature j
    jvals = np.concatenate([st, ac], axis=1).astype(np.float16)  # [B, 17]
    stac16w[:, 128:672] = np.repeat(jvals, NS, axis=1)
    stac16w[:, 672] = 1.0
    return dict(blob16=blob16, blobf=blobf), stac16w


def _core_inputs(inputs, common=None):
    """Build the 8 per-core input maps from the full problem inputs."""
    if common is None:
        common, stac16w = _host_prep(inputs)
    else:
        common, stac16w = common
    state = np.ascontiguousarray(np.asarray(inputs["state"], np.float32))
    in_maps = []
    for i in range(N_CORES):
        m = dict(common)
        m["state"] = state[i * RPC:(i + 1) * RPC]
        m["stac16w"] = stac16w[i * RPC:(i + 1) * RPC]
        in_maps.append(m)
    return in_maps


_NC_CACHE = {}


def _get_nc(rpc=RPC):
    if rpc not in _NC_CACHE:
        _NC_CACHE[rpc] = build_nc(rpc)
    return _NC_CACHE[rpc]


def kernel(**inputs):
    from concourse.bass_utils import run_bass_kernel_spmd

    nc = _get_nc()
    in_maps = _core_inputs(inputs)
    res = run_bass_kernel_spmd(nc, in_maps, list(range(N_CORES)))
    return np.concatenate([r["out"] for r in res.results], axis=0)
